# revision 2
# baseline (speedup 1.0000x reference)
"""Sliding-window attention (L=4096, H=2048, 16 heads, window 1024) on 8 TRN2 cores.

Sequence sharding with NON-redundant K/V projection: core c owns rows
[512c, 512c+512) and projects K/V only for those rows (all 16 heads), then
chunked AllGathers (2 for K, 2 for V, 8 heads each) distribute them; each
core reads back its 1536-row window (slots c-2, c-1, c of the gathered
buffer, with 2 zeroed front-pad slots so the runtime offset pid*SLOT never
goes negative). Q projection + attention are interleaved per head so the
collectives hide behind compute. Attention uses j-major S^T tiles (one
matmul + one exp per window k-tile, up to 512 q columns wide) with pad
masking folded into the exp bias (per-core tselj data).

All matmuls run in bf16 (fp32 PSUM accumulation).
"""

import sys

import numpy as np

if "/opt/trn_rl_repo" not in sys.path:
    sys.path.insert(0, "/opt/trn_rl_repo")

L = 4096
H = 2048
NH = 16
D = 128
WIN = 1024
NCORES = 8
QROWS = L // NCORES          # 512 query rows per core
NQT = QROWS // 128           # 4 q tiles per core
NWT = 12                     # window k tiles per q-tile-group (3 slots x 4)
NKT = 9                      # k tiles attended per q tile
PADS = 2                     # front pad slots in gathered buffers
CH = 8                       # heads per collective chunk
CHUNK = D * QROWS            # elems per [128, 512] chunk
SLOT = 2 * CH * CHUNK        # per-rank slot in t_kv: 8 k-chunks + 8 v-chunks
ROPE_THETA = 10000.0
SCALE = float(D) ** -0.5
NEG = -1e30

_CACHE = {}


def _trace(tc, aps, ccs):
    from contextlib import ExitStack

    from concourse import mybir
    from concourse.ap import AP

    nc = tc.nc
    f32 = mybir.dt.float32
    bf16 = mybir.dt.bfloat16
    AF = mybir.ActivationFunctionType
    hs, wq, wk, wv, wo, cosw, sinw, tselj, maskl, maskd, idb, out = aps
    cc_kv, t_kv = ccs

    ctx = ExitStack()
    const = ctx.enter_context(tc.tile_pool(name="const", bufs=1))
    hstp = ctx.enter_context(tc.tile_pool(name="hst", bufs=1))
    qtp = ctx.enter_context(tc.tile_pool(name="qtp", bufs=1))
    otp = ctx.enter_context(tc.tile_pool(name="otp", bufs=1))
    win = ctx.enter_context(tc.tile_pool(name="win", bufs=1))
    wstr = ctx.enter_context(tc.tile_pool(name="wstr", bufs=2))
    kvp = ctx.enter_context(tc.tile_pool(name="kvp", bufs=3))
    rope = ctx.enter_context(tc.tile_pool(name="rope", bufs=3))
    attn = ctx.enter_context(tc.tile_pool(name="attn", bufs=3))
    phc = ctx.enter_context(tc.tile_pool(name="phc", bufs=2))
    ps_o = ctx.enter_context(tc.tile_pool(name="ps_o", bufs=4, space="PSUM"))
    ps_t = ctx.enter_context(tc.tile_pool(name="ps_t", bufs=1, space="PSUM"))
    ps_b = ctx.enter_context(tc.tile_pool(name="ps_b", bufs=3, space="PSUM"))

    # ---- constants ----
    maskl_sb = const.tile([128, 128], f32, name="maskl_sb")
    nc.sync.dma_start(out=maskl_sb, in_=maskl)
    maskd_sb = const.tile([128, 128], f32, name="maskd_sb")
    nc.sync.dma_start(out=maskd_sb, in_=maskd)
    idb_sb = const.tile([128, 128], bf16, name="idb_sb")
    nc.sync.dma_start(out=idb_sb, in_=idb)
    cos_sb = const.tile([128, QROWS], bf16, name="cos_sb")
    nc.sync.dma_start(out=cos_sb, in_=cosw)
    sin_sb = const.tile([128, QROWS], bf16, name="sin_sb")
    nc.sync.dma_start(out=sin_sb, in_=sinw)
    tselj_sb = const.tile([128, NWT], f32, name="tselj_sb")
    nc.sync.dma_start(out=tselj_sb, in_=tselj)

    pid = nc.sync.partition_id()
    pid_gp = nc.gpsimd.partition_id()

    # window ring buffers; memset once. On cores 0/1 the out-of-bounds slot
    # reads are runtime-skipped, so these zeros persist as the causal pad.
    NR = 3
    kwb = [win.tile([128, NWT * 128], bf16, name=f"kwb{r}") for r in range(NR)]
    vwb = [win.tile([128, NWT, 130], bf16, name=f"vwb{r}") for r in range(NR)]
    for r in range(NR):
        nc.vector.memset(kwb[r], 0.0)
        nc.vector.memset(vwb[r], 0.0)
        nc.vector.memset(vwb[r][:, :, 128:129], 1.0)

    # attention outputs, transposed: [feat-part, head, q-tile, row]
    ot_sb = otp.tile([128, NH, NQT, 128], bf16, name="ot_sb")

    # ---- load own hs rows, transpose to hsT via PE (kt-major so K proj
    # can start consuming feature chunks before the whole transpose ends) ----
    hsT = hstp.tile([128, 16, QROWS], bf16, name="hsT")
    with tc.tile_pool(name="hsl", bufs=1) as hsl:
        hrow = {}
        for half in range(2):
            for rt in range(NQT):
                hp = hsl.tile([128, 1024], bf16, name=f"hrow{rt}_{half}")
                nc.sync.dma_start(
                    out=hp,
                    in_=hs[rt * 128:(rt + 1) * 128,
                           half * 1024:(half + 1) * 1024],
                )
                hrow[rt * 2 + half] = hp

        for kt in range(16):
            for rt in range(NQT):
                tp = ps_t.tile([128, 128], bf16, tag="t", name=f"htp{rt}_{kt}")
                nc.tensor.transpose(
                    tp,
                    hrow[rt * 2 + kt // 8][:, (kt % 8) * 128:(kt % 8 + 1) * 128],
                    idb_sb,
                )
                nc.vector.tensor_copy(
                    out=hsT[:, kt, rt * 128:(rt + 1) * 128], in_=tp)

    def rope_pair(dst, src_ps):
        """RoPE over own rows: dst[d,r] = src[d,r]*cos[d,r] + src[(d+64)%128,r]*sin[d,r]."""
        qbf = rope.tile([128, QROWS], bf16, tag="qbf")
        nc.scalar.copy(qbf, src_ps)
        qsw = rope.tile([128, QROWS], bf16, tag="qsw")
        nc.scalar.dma_start(out=qsw[0:64, :], in_=qbf[64:128, :])
        nc.scalar.dma_start(out=qsw[64:128, :], in_=qbf[0:64, :])
        t1 = rope.tile([128, QROWS], bf16, tag="t1")
        nc.vector.tensor_mul(t1, qbf, cos_sb)
        t2 = rope.tile([128, QROWS], bf16, tag="t2")
        nc.vector.tensor_mul(t2, qsw, sin_sb)
        nc.vector.tensor_add(dst, t1, t2)

    def stream_w(w_dram, h, wtag):
        w_b = wstr.tile([128, 16, 128], bf16, tag=wtag, bufs=4)
        nc.sync.dma_start(
            out=w_b,
            in_=w_dram[:, h * 128:(h + 1) * 128]
            .rearrange("(kt p) f -> p kt f", p=128),
        )
        return w_b

    def k_heads(i):
        """K projection + RoPE for heads [8i, 8i+8) -> cc_kv[i] k section."""
        for hh in range(CH):
            h = i * CH + hh
            wk_h = stream_w(wk, h, "wk_h")
            ps = ps_b.tile([128, QROWS], f32, tag="b", name=f"kp{h}")
            for kt in range(16):
                nc.tensor.matmul(
                    ps, lhsT=wk_h[:, kt, :], rhs=hsT[:, kt, :],
                    start=(kt == 0), stop=(kt == 15),
                )
            kr = kvp.tile([128, QROWS], bf16, tag="kr")
            rope_pair(kr, ps)
            nc.sync.dma_start(out=cc_kv[i][hh], in_=kr)

    def v_heads(i):
        """V projection for heads [8i, 8i+8) -> cc_kv[i] v section, then AG."""
        for nb in (2 * i, 2 * i + 1):
            wv_nb = wstr.tile([128, 16, 512], bf16, tag="wv_nb")
            nc.scalar.dma_start(
                out=wv_nb,
                in_=wv[:, nb * 512:(nb + 1) * 512]
                .rearrange("(kt p) f -> p kt f", p=128),
            )
            for jt in range(NQT):
                ps = ps_b.tile([128, 512], f32, tag="b", name=f"vp{nb}_{jt}")
                for kt in range(16):
                    nc.tensor.matmul(
                        ps, lhsT=hsT[:, kt, jt * 128:(jt + 1) * 128],
                        rhs=wv_nb[:, kt, :],
                        start=(kt == 0), stop=(kt == 15),
                    )
                vs = kvp.tile([128, 512], bf16, tag="vs")
                nc.scalar.copy(vs, ps)
                hb = (nb % 2) * 4
                # v chunk for head g is [128r, (4jt x 128d)] at cc_kv[CH+g]
                nc.scalar.dma_start(
                    out=cc_kv[i][CH + hb:CH + hb + 4, :,
                                 jt * 128:(jt + 1) * 128].transpose([1, 0, 2]),
                    in_=vs,
                )
        nc.gpsimd.collective_compute(
            "AllGather", mybir.AluOpType.bypass,
            replica_groups=[list(range(NCORES))],
            ins=[cc_kv[i].opt()],
            outs=[t_kv[i].opt()],
        )

    # interleave K/V half-chunks so the 2 serial AllGathers start early and
    # pipeline behind the remaining projection compute
    k_heads(0)
    v_heads(0)
    k_heads(1)
    v_heads(1)

    # ---- Q projection + RoPE for all heads (tensor work that overlaps AGs) ----
    qT = qtp.tile([128, NH, QROWS], bf16, name="qT")
    for h in range(NH):
        wq_h = stream_w(wq, h, "wq_h")
        ps = ps_b.tile([128, QROWS], f32, tag="b", name=f"qp{h}")
        for kt in range(16):
            nc.tensor.matmul(
                ps, lhsT=wq_h[:, kt, :], rhs=hsT[:, kt, :],
                start=(kt == 0), stop=(kt == 15),
            )
        rope_pair(qT[:, h, :], ps)

    # ---- window fetch (ring, prefetched) ----
    def fetch_window(h):
        i, hh = h // CH, h % CH
        kw, vw = kwb[h % NR], vwb[h % NR]
        sap = t_kv[i][0]  # [128, 512] template AP
        for s in range(3):
            nc.gpsimd.dma_start(
                out=kw[:, s * 512:(s + 1) * 512],
                in_=AP(
                    tensor=sap.tensor,
                    offset=(pid_gp + (s - 2)) * SLOT + hh * CHUNK,
                    ap=sap.ap,
                    dep_tracking_offset=0,
                ),
                bounds_check="skip_entire_dma",
            )
        for s in range(3):
            nc.gpsimd.dma_start(
                out=vw[:, s * 4:(s + 1) * 4, 0:128],
                in_=AP(
                    tensor=sap.tensor,
                    offset=(pid_gp + (s - 2)) * SLOT + (CH + hh) * CHUNK,
                    ap=sap.ap,
                    dep_tracking_offset=0,
                ),
                bounds_check="skip_entire_dma",
            )
        return kw, vw

    # ---- o_proj in two kt-halves: pass 0 (heads 0-7) runs inside the
    # second AllGather's wait bubble; pass 1 adds heads 8-15 and stores.
    o_part = otp.tile([128, NQT, H], bf16, name="o_part")

    def o_proj_pass(second):
        ktr = range(8, 16) if second else range(0, 8)
        for lqp in range(NQT // 2):
            for nb in range(4):
                pss = [
                    ps_b.tile([128, 512], f32, tag="b",
                              name=f"op{int(second)}_{lqp}_{nb}_{i}")
                    for i in range(2)
                ]
                for kt in ktr:
                    wos = phc.tile([128, 512], bf16, tag="wos", bufs=10)
                    nc.sync.dma_start(
                        out=wos,
                        in_=wo[kt * 128:(kt + 1) * 128,
                               nb * 512:(nb + 1) * 512],
                    )
                    for i in range(2):
                        nc.tensor.matmul(
                            pss[i], lhsT=ot_sb[:, kt, lqp * 2 + i, :], rhs=wos,
                            start=(kt == ktr[0]), stop=(kt == ktr[-1]),
                        )
                for i in range(2):
                    lq = lqp * 2 + i
                    dst = o_part[:, lq, nb * 512:(nb + 1) * 512]
                    if not second:
                        nc.vector.tensor_copy(out=dst, in_=pss[i])
                    else:
                        ob = phc.tile([128, 512], f32, tag="ob", bufs=3)
                        nc.vector.tensor_add(ob, pss[i], dst)
                        nc.sync.dma_start(
                            out=out[lq, :, nb * 512:(nb + 1) * 512], in_=ob
                        )

    fetch_window(0)
    fetch_window(1)

    # ---- attention per head ----
    for h in range(NH):
        if h == CH:
            o_proj_pass(second=False)
        kw, vw = kwb[h % NR], vwb[h % NR]
        if h + 2 < NH:
            fetch_window(h + 2)

        o_ps = [
            ps_o.tile([128, 129], f32, tag="o", name=f"o{h}_{lq}")
            for lq in range(NQT)
        ]
        for j in range(NWT):
            lq_lo = max(0, j - 8)
            lq_hi = min(NQT - 1, j)
            ncols = (lq_hi - lq_lo + 1) * 128
            st_full = ps_b.tile([128, 512], f32, tag="b", name=f"st{h}_{j}")
            st_ps = st_full[:, 0:ncols]
            nc.tensor.matmul(
                st_ps, lhsT=kw[:, j * 128:(j + 1) * 128],
                rhs=qT[:, h, lq_lo * 128:(lq_hi + 1) * 128],
                start=True, stop=True,
            )
            if j <= 3:
                nc.vector.tensor_add(
                    st_ps[:, (j - lq_lo) * 128:(j - lq_lo + 1) * 128],
                    st_ps[:, (j - lq_lo) * 128:(j - lq_lo + 1) * 128],
                    maskl_sb,
                )
            if j >= 8:
                nc.vector.tensor_add(
                    st_ps[:, 0:128], st_ps[:, 0:128], maskd_sb,
                )
            p_full = attn.tile([128, 512], bf16, tag="p_sb")
            p_sb = p_full[:, 0:ncols]
            nc.scalar.activation(
                p_sb, st_ps, AF.Exp,
                bias=tselj_sb[:, j:j + 1], scale=SCALE,
            )
            for lq in range(lq_lo, lq_hi + 1):
                nc.tensor.matmul(
                    o_ps[lq][:, 0:129],
                    lhsT=p_sb[:, (lq - lq_lo) * 128:(lq - lq_lo + 1) * 128],
                    rhs=vw[:, j, 0:129],
                    start=(j == lq), stop=(j == lq + 8),
                )
        for lq in range(NQT):
            rinv = attn.tile([128, 1], f32, tag="rinv")
            nc.vector.reciprocal(rinv, o_ps[lq][:, 128:129])
            ao = attn.tile([128, 128], bf16, tag="ao")
            nc.vector.tensor_scalar_mul(ao, o_ps[lq][:, 0:128], rinv)
            tp = ps_t.tile([128, 128], bf16, tag="t", name=f"aot{h}_{lq}")
            nc.tensor.transpose(tp, ao, idb_sb)
            nc.vector.tensor_copy(out=ot_sb[:, h, lq, :], in_=tp)

    o_proj_pass(second=True)

    ctx.close()


def _build(timing=False):
    """Build the module. With timing=True, all real tensors become Internal
    DRAM (garbage contents, valid timing) and tiny dummy ExternalInput/Output
    tensors are added, so benchmarking excludes host<->device transfer."""
    import concourse.bacc as bacc
    import concourse.tile as tile
    from concourse import mybir

    f32 = mybir.dt.float32
    bf16 = mybir.dt.bfloat16

    nc = bacc.Bacc("TRN2", target_bir_lowering=False, debug=False,
                   num_devices=NCORES)
    kind = {} if timing else {"kind": "ExternalInput"}
    okind = {} if timing else {"kind": "ExternalOutput"}
    aps = [
        nc.dram_tensor("hs", [QROWS, H], bf16, **kind).ap(),
        nc.dram_tensor("wq", [H, H], bf16, **kind).ap(),
        nc.dram_tensor("wk", [H, H], bf16, **kind).ap(),
        nc.dram_tensor("wv", [H, H], bf16, **kind).ap(),
        nc.dram_tensor("wo", [H, H], bf16, **kind).ap(),
        nc.dram_tensor("cosw", [D, QROWS], bf16, **kind).ap(),
        nc.dram_tensor("sinw", [D, QROWS], bf16, **kind).ap(),
        nc.dram_tensor("tselj", [128, NWT], f32, **kind).ap(),
        nc.dram_tensor("maskl", [128, 128], f32, **kind).ap(),
        nc.dram_tensor("maskd", [128, 128], f32, **kind).ap(),
        nc.dram_tensor("idb", [128, 128], bf16, **kind).ap(),
        nc.dram_tensor("out", [NQT, 128, H], f32, **okind).ap(),
    ]
    cc_kv = [
        nc.dram_tensor(f"cc_kv{i}", [2 * CH, D, QROWS], bf16).ap()
        for i in range(2)
    ]
    t_kv = [
        nc.dram_tensor(
            f"t_kv{i}", [NCORES * 2 * CH, D, QROWS], bf16,
            addr_space="Shared",
        ).ap()
        for i in range(2)
    ]
    dummies = None
    if timing:
        dummies = (
            nc.dram_tensor("dummy_in", [1, 8], f32, kind="ExternalInput").ap(),
            nc.dram_tensor("dummy_out", [1, 8], f32, kind="ExternalOutput").ap(),
        )
    with tile.TileContext(nc) as tc:
        _trace(tc, aps, (cc_kv, t_kv))
        if dummies is not None:
            with tc.tile_pool(name="dummy", bufs=1) as dp:
                dt_ = dp.tile([1, 8], f32, name="dummy_sb")
                nc.sync.dma_start(out=dt_, in_=dummies[0])
                nc.sync.dma_start(out=dummies[1], in_=dt_)
    nc.compile()
    return nc


def bench_device(iters=50):
    """Marginal per-iteration time of the compute with dummy-sized I/O."""
    if "timing_runner" not in _CACHE:
        tnc = _build(timing=True)
        _CACHE["timing_runner"] = _Runner(tnc)
    r = _CACHE["timing_runner"]
    maps = [{"dummy_in": np.zeros((1, 8), np.float32)} for _ in range(NCORES)]
    return r.bench(maps, iters=iters)


def _host_constants():
    import ml_dtypes

    inv = 1.0 / (ROPE_THETA ** (np.arange(0, D, 2, dtype=np.float64) / D))
    ii = np.arange(128)
    # masks for S^T [j, i] tiles; valid -> 0, invalid -> NEG
    maskl = np.where(ii[:, None] > ii[None, :], 0.0, NEG).astype(np.float32)
    maskd = np.where(ii[:, None] <= ii[None, :], 0.0, NEG).astype(np.float32)
    idb = np.eye(128).astype(ml_dtypes.bfloat16)

    cos_list, sin_list, tselj_list = [], [], []
    for c in range(NCORES):
        pos = np.arange(c * QROWS, (c + 1) * QROWS, dtype=np.float64)
        ang = inv[:, None] * pos[None, :]  # [64, QROWS]
        cos_list.append(np.concatenate([np.cos(ang), np.cos(ang)], 0)
                        .astype(ml_dtypes.bfloat16))
        sin_list.append(np.concatenate([-np.sin(ang), np.sin(ang)], 0)
                        .astype(ml_dtypes.bfloat16))
        # tselj[j] = 0 if window k-tile j is a real (non-pad) tile else NEG
        ts = np.zeros((NWT,), np.float32)
        for j in range(NWT):
            if 4 * c - 8 + j < 0:
                ts[j] = NEG
        tselj_list.append(np.broadcast_to(ts, (128, NWT)).copy())
    return cos_list, sin_list, tselj_list, maskl, maskd, idb


def _get_state():
    if "nc" not in _CACHE:
        _CACHE["nc"] = _build()
        _CACHE["consts"] = _host_constants()
    return _CACHE["nc"], _CACHE["consts"]


def _in_maps(hidden_states, wq, wk, wv, wo, consts):
    import ml_dtypes

    bf16 = ml_dtypes.bfloat16
    hs = np.asarray(hidden_states, np.float32).reshape(L, H).astype(bf16)
    wq = np.asarray(wq, np.float32).astype(bf16)
    wk = np.asarray(wk, np.float32).astype(bf16)
    wv = np.asarray(wv, np.float32).astype(bf16)
    wo = np.asarray(wo, np.float32).astype(bf16)
    cos_list, sin_list, tselj_list, maskl, maskd, idb = consts
    maps = []
    for c in range(NCORES):
        maps.append({
            "hs": hs[c * QROWS:(c + 1) * QROWS],
            "wq": wq,
            "wk": wk,
            "wv": wv,
            "wo": wo,
            "cosw": cos_list[c],
            "sinw": sin_list[c],
            "tselj": tselj_list[c],
            "maskl": maskl,
            "maskd": maskd,
            "idb": idb,
        })
    return maps


def _gather(results):
    full = np.empty((L, H), np.float32)
    for c in range(NCORES):
        full[c * QROWS:(c + 1) * QROWS] = results[c]["out"].reshape(QROWS, H)
    return full.reshape(1, L, H)


class _Runner:
    """Persistent jitted shard_map executable over the 8 axon cores."""

    def __init__(self, nc):
        import jax
        from jax.sharding import Mesh, PartitionSpec
        from jax.experimental.shard_map import shard_map
        from concourse import mybir
        from concourse import bass2jax

        bass2jax.install_neuronx_cc_hook()

        partition_name = (
            nc.partition_id_tensor.name if nc.partition_id_tensor else None
        )
        in_names, out_names, out_avals, zero_outs = [], [], [], []
        for alloc in nc.m.functions[0].allocations:
            if not isinstance(alloc, mybir.MemoryLocationSet):
                continue
            name = alloc.memorylocations[0].name
            if alloc.kind == "ExternalInput":
                if name != partition_name:
                    in_names.append(name)
            elif alloc.kind == "ExternalOutput":
                out_names.append(name)
                shape = tuple(alloc.tensor_shape)
                dtype = mybir.dt.np(alloc.dtype)
                out_avals.append(jax.core.ShapedArray(shape, dtype))
                zero_outs.append(np.zeros(shape, dtype))
        self.n_params = len(in_names)
        self.in_names = list(in_names)
        self.out_names = out_names
        all_names = in_names + out_names
        if partition_name is not None:
            all_names = all_names + [partition_name]

        def _body(*args):
            operands = list(args)
            if partition_name is not None:
                operands.append(bass2jax.partition_id_tensor())
            outs = bass2jax._bass_exec_p.bind(
                *operands,
                out_avals=tuple(out_avals),
                in_names=tuple(all_names),
                out_names=tuple(out_names),
                lowering_input_output_aliases=(),
                sim_require_finite=True,
                sim_require_nnan=True,
                nc=nc,
            )
            return tuple(outs)

        devices = jax.devices()[:NCORES]
        assert len(devices) == NCORES
        self.mesh = Mesh(np.asarray(devices), ("core",))
        in_specs = (PartitionSpec("core"),) * (self.n_params + len(out_names))
        out_specs = (PartitionSpec("core"),) * len(out_names)
        self.sharded = jax.jit(
            shard_map(_body, mesh=self.mesh, in_specs=in_specs,
                      out_specs=out_specs, check_rep=False),
            keep_unused=True,
        )
        self.out_avals = out_avals
        self.concat_zeros = [
            np.zeros((NCORES * z.shape[0], *z.shape[1:]), z.dtype)
            for z in zero_outs
        ]

    def pack(self, maps):
        return [
            np.concatenate([np.asarray(maps[c][n]) for c in range(NCORES)], axis=0)
            for n in self.in_names
        ]

    def run(self, maps):
        concat_in = self.pack(maps)
        out_arrs = self.sharded(*concat_in, *self.concat_zeros)
        return [
            {
                n: np.asarray(out_arrs[i]).reshape(
                    NCORES, *self.out_avals[i].shape)[c]
                for i, n in enumerate(self.out_names)
            }
            for c in range(NCORES)
        ]

    def bench(self, maps, iters=10):
        """Time repeated executions with inputs resident on device."""
        import time

        import jax

        args = [jax.device_put(a) for a in self.pack(maps)]
        args += [jax.device_put(z) for z in self.concat_zeros]
        out = self.sharded(*args)  # warm
        jax.block_until_ready(out)
        t0 = time.perf_counter()
        for _ in range(iters):
            out = self.sharded(*args)
        jax.block_until_ready(out)
        return (time.perf_counter() - t0) / iters


def _get_runner():
    nc, consts = _get_state()
    if "runner" not in _CACHE:
        _CACHE["runner"] = _Runner(nc)
    return _CACHE["runner"], consts


def kernel(hidden_states, wq, wk, wv, wo):
    runner, consts = _get_runner()
    maps = _in_maps(hidden_states, wq, wk, wv, wo, consts)
    return _gather(runner.run(maps))


def bench(hidden_states, wq, wk, wv, wo, iters=10):
    runner, consts = _get_runner()
    maps = _in_maps(hidden_states, wq, wk, wv, wo, consts)
    return runner.bench(maps, iters=iters)
